# revision 1
# baseline (speedup 1.0000x reference)
"""CBOW negative-sampling loss kernel for Trainium2 (8 NeuronCores, SPMD).

Per batch element b: gather 21 rows of 50 floats (10 ctx rows from in_embed,
1 pos + 10 neg from out_embed), context sum, 11 dot products, log-sigmoids,
global mean.

This runtime's DGE supports only *scalar* dynamic offsets (one offset per
dest partition-row) for indirect DMA - vector-offset gathers scramble, and
InstDMAGatherAnt fails at runtime here.  So the kernel issues one
indirect_dma_start per (tile, j): a [128,1] offset column gathers one table
row per partition into g[:, j*50:(j+1)*50].  21 gathers per 128-element
tile, 2688 per core - SWDGE-overhead-bound but correct.

Host: concat tables into one fp16 [100000, 50] (out_embed offset by VOCAB),
indices as int32 [128, ntiles*21] with batch element t*128+p on partition p.
Scores: pos negated via the reduce negate flag so one sigmoid(-0.1*x) +
ln(x+1e-10) + accum_out pass yields the per-partition loss sums.
Host: loss = -(sum of partials) / B.
"""

import sys

import numpy as np

if "/opt/trn_rl_repo" not in sys.path:
    sys.path.insert(0, "/opt/trn_rl_repo")

from concourse import bass, mybir  # noqa: E402
from concourse import bass_utils  # noqa: E402
from concourse import tile  # noqa: E402
from concourse.bacc import Bacc  # noqa: E402

VOCAB = 50000
DIM = 50
B = 131072
CTX = 10
NEG = 10
NIDX = CTX + 1 + NEG  # 21 rows per batch element: [ctx*10, pos, neg*10]
EPS = 1e-10

NCORES = 8
P = 128
BC = B // NCORES  # 16384
NTILES = BC // P  # 128

f16 = mybir.dt.float16
f32 = mybir.dt.float32


def build_nc(ntiles: int = NTILES, repeats: int = 1, dump_scores: bool = False):
    nc = Bacc(None, target_bir_lowering=False)
    eps_t = nc.alloc_sbuf_tensor("const-eps", [P, 1], f32)
    nc.gpsimd.memset(eps_t.ap(), EPS)
    nc.const_aps.aps[(f32, EPS)] = eps_t.ap()
    nc.all_engine_barrier()

    table = nc.dram_tensor("table", [2 * VOCAB, DIM], f16, kind="ExternalInput")
    idx = nc.dram_tensor(
        "idx", [P, ntiles * NIDX], mybir.dt.int32, kind="ExternalInput"
    )
    partial = nc.dram_tensor("partial", [P, 1], f32, kind="ExternalOutput")
    scores_out = (
        nc.dram_tensor("scores_out", [P, ntiles * 11], f32, kind="ExternalOutput")
        if dump_scores
        else None
    )

    with tile.TileContext(nc) as tc:
        with (
            tc.tile_pool(name="idxp", bufs=1) as ipool,
            tc.tile_pool(name="gather", bufs=3) as gpool,
            tc.tile_pool(name="work", bufs=2) as wpool,
            tc.tile_pool(name="stage", bufs=1) as spool,
        ):
          for rep in range(repeats):
            it = ipool.tile([P, ntiles * NIDX], mybir.dt.int32, tag="it")
            nc.sync.dma_start(out=it[:], in_=idx[:])
            itv = it[:].rearrange("p (t j) -> p t j", t=ntiles, j=NIDX)

            scores = spool.tile([P, ntiles * 11], f32, tag="scores")
            sv = scores[:].rearrange("p (t j) -> p t j", t=ntiles, j=11)

            for t in range(ntiles):
                g = gpool.tile([P, NIDX * DIM], f16, tag="g")
                g3 = g[:].rearrange("p (j d) -> p j d", j=NIDX, d=DIM)
                # 21 per-partition-scalar gathers: one table row per partition
                for j in range(NIDX):
                    nc.gpsimd.indirect_dma_start(
                        out=g3[:, j, :],
                        out_offset=None,
                        in_=table[:],
                        in_offset=bass.IndirectOffsetOnAxis(
                            ap=itv[:, t, j : j + 1], axis=0
                        ),
                    )
                # context tree-sum over rows 0..9
                s1 = wpool.tile([P, 5 * DIM], f16, tag="s1")
                s1v = s1[:].rearrange("p (k d) -> p k d", k=5, d=DIM)
                nc.vector.tensor_add(
                    out=s1v, in0=g3[:, 0:5, :], in1=g3[:, 5:10, :]
                )
                s2 = wpool.tile([P, 2 * DIM], f16, tag="s2")
                s2v = s2[:].rearrange("p (k d) -> p k d", k=2, d=DIM)
                nc.vector.tensor_add(
                    out=s2v, in0=s1v[:, 0:2, :], in1=s1v[:, 2:4, :]
                )
                s3 = wpool.tile([P, DIM], f16, tag="s3")
                nc.vector.tensor_add(
                    out=s3[:], in0=s2v[:, 0, :], in1=s2v[:, 1, :]
                )
                ctx = wpool.tile([P, DIM], f16, tag="ctx")
                nc.vector.tensor_add(out=ctx[:], in0=s3[:], in1=s1v[:, 4, :])

                # raw scores for rows 10..20 ([pos, neg*10])
                prod = wpool.tile([P, 11 * DIM], f16, tag="prod")
                prodv = prod[:].rearrange("p (j d) -> p j d", j=11, d=DIM)
                for j in range(11):
                    nc.vector.tensor_mul(
                        out=prodv[:, j, :], in0=g3[:, 10 + j, :], in1=ctx[:]
                    )
                nc.vector.tensor_reduce(
                    out=sv[:, t, 0:1],
                    in_=prodv[:, 0:1, :],
                    axis=mybir.AxisListType.X,
                    op=mybir.AluOpType.add,
                    negate=True,
                )
                nc.vector.tensor_reduce(
                    out=sv[:, t, 1:11],
                    in_=prodv[:, 1:11, :],
                    axis=mybir.AxisListType.X,
                    op=mybir.AluOpType.add,
                    negate=False,
                )

            acc = spool.tile([P, 1], f32, tag="acc")
            if dump_scores:
                nc.sync.dma_start(out=scores_out[:], in_=scores[:])
            nc.scalar.activation(
                out=scores[:],
                in_=scores[:],
                func=mybir.ActivationFunctionType.Sigmoid,
                scale=-0.1,
            )
            nc.scalar.activation(
                out=scores[:],
                in_=scores[:],
                func=mybir.ActivationFunctionType.Ln,
                bias=EPS,
                accum_out=acc[:],
            )
            nc.sync.dma_start(out=partial[:], in_=acc[:])

    nc.compile()
    return nc


def _prep_inputs(context_idxs, pos_target, neg_samples, in_embed_W, out_embed_W):
    idx_all = np.concatenate(
        [
            np.asarray(context_idxs, dtype=np.int64),
            np.asarray(pos_target, dtype=np.int64)[:, None] + VOCAB,
            np.asarray(neg_samples, dtype=np.int64) + VOCAB,
        ],
        axis=1,
    ).astype(np.int32)  # [B, 21] = [ctx*10, pos, neg*10]
    table = np.concatenate(
        [np.asarray(in_embed_W), np.asarray(out_embed_W)], axis=0
    ).astype(np.float16)

    in_maps = []
    for c in range(NCORES):
        sl = idx_all[c * BC : (c + 1) * BC]
        idx_c = (
            sl.reshape(NTILES, P, NIDX)
            .transpose(1, 0, 2)
            .reshape(P, NTILES * NIDX)
            .copy()
        )
        in_maps.append({"table": table, "idx": idx_c})
    return in_maps


def kernel(context_idxs, pos_target, neg_samples, in_embed_W, out_embed_W):
    in_maps = _prep_inputs(
        context_idxs, pos_target, neg_samples, in_embed_W, out_embed_W
    )
    nc = build_nc()
    res = bass_utils.run_bass_kernel_spmd(nc, in_maps, core_ids=list(range(NCORES)))
    total = sum(float(r["partial"].sum()) for r in res.results)
    return np.float32(-total / B)

